# revision 1
# baseline (speedup 1.0000x reference)
"""Trainium2 Bass kernel for the distributed CLIP-style contrastive loss.

loss = 0.5 * ( mean_i( LSE_row(i) - diag(i) ) + mean_j( LSE_col(j) - diag(j) ) )
with logits = tau * ftir @ raman.T, tau = min(exp(log_tau), 100), B=4096, D=512.

Sharding: rows of the [B, B] logits matrix are split across 8 cores (512 rows
each).  Each core computes BOTH its row-slab of logits (ftir_shard @ raman.T)
and its row-slab of logits.T (raman_shard @ ftir.T), so the column-softmax is
just a second row-softmax and no collectives are needed.  Row log-sum-exp is
computed with an exact two-level scheme: per 1024-wide block the VectorE takes
the block max straight out of PSUM (negated, as the exp bias), the ScalarE
computes exp(x - m_b) with a fused free-dim accumulation (accum_out), and the
host combines block stats exactly: LSE = M + log(sum_b s_b * exp(m_b - M)).

Each core returns raw per-block stats (negm/sums, [128, 32]) and the diagonal
dot products ([1, 512]); the host does the exact two-level LSE combine and the
final scalar reduction in float64.
"""

import sys

import numpy as np

for _p in ("/opt/trn_rl_repo", "/root/.axon_site/_ro/trn_rl_repo"):
    if _p not in sys.path:
        sys.path.append(_p)

from contextlib import ExitStack

import concourse.bacc as bacc
import concourse.tile as tile
from concourse import mybir
from concourse.bass_utils import run_bass_kernel_spmd

B = 4096
D = 512
NCORES = 8
SH = B // NCORES  # 512 rows per core
P = 128
KC = D // P  # 4 k-chunks of 128
MT = SH // P  # 4 m-tiles of 128 rows
BLK = 1024  # PSUM stats-block width
NB = B // BLK  # 4 blocks per row
SUB = 512  # matmul N per instruction
CHW = 2048  # DMA chunk width for the full tensors
NCH = B // CHW  # 2 chunks per k-slice

# matmul input dtype: bfloat16 (fast, half DMA) or float32r (full-rate fp32
# streaming mode) or float32 (4x slower matmul).
DT_IN = mybir.dt.bfloat16

F32 = mybir.dt.float32
AX = mybir.AxisListType
ALU = mybir.AluOpType
ACTF = mybir.ActivationFunctionType

# toggled by test harness for profiling
PROFILE = False
LAST_RESULTS = None

_prog_cache = {}


def _build_program(dt_in):
    nc = bacc.Bacc(
        "TRN2",
        target_bir_lowering=False,
        debug=False,
        enable_partition_id=False,
        enable_asserts=False,
    )

    ats = nc.dram_tensor("ats", [D, SH], dt_in, kind="ExternalInput").ap()
    bts = nc.dram_tensor("bts", [D, SH], dt_in, kind="ExternalInput").ap()
    atf = nc.dram_tensor("atf", [D, B], dt_in, kind="ExternalInput").ap()
    btf = nc.dram_tensor("btf", [D, B], dt_in, kind="ExternalInput").ap()
    negm_out = nc.dram_tensor("negm", [P, 2 * MT * NB], F32, kind="ExternalOutput").ap()
    sums_out = nc.dram_tensor("sums", [P, 2 * MT * NB], F32, kind="ExternalOutput").ap()
    diag_out = nc.dram_tensor("diag", [1, SH], F32, kind="ExternalOutput").ap()

    with ExitStack() as ctx:
        tc = ctx.enter_context(tile.TileContext(nc))
        inp = ctx.enter_context(tc.tile_pool(name="inp", bufs=1))
        psum = ctx.enter_context(tc.tile_pool(name="psum", bufs=3, space="PSUM"))
        dpsum = ctx.enter_context(tc.tile_pool(name="dpsum", bufs=1, space="PSUM"))
        scr = ctx.enter_context(tc.tile_pool(name="scr", bufs=3))
        stats = ctx.enter_context(tc.tile_pool(name="stats", bufs=2))
        small = ctx.enter_context(tc.tile_pool(name="small", bufs=2))

        # ---- PE warm-up: dummy matmuls while input DMAs stream in. ----
        # Keeps TensorE busy through the DMA-bound head so HAM reaches
        # K=8/8 before the first real matmul (else ~25 MMs run at 1.2GHz).
        warm_sb = inp.tile([P, SUB], dt_in, tag="warm_sb")
        nc.vector.memset(warm_sb, 0.0)
        # dummy exp primes the ACT Exp table during the DMA-bound head —
        # otherwise the lazy ACT_TABLE_LOAD (1.28us) lands right before the
        # first real exp and delays the first PSUM release.
        warm_act = inp.tile([P, 1], F32, tag="warm_act")
        nc.scalar.activation(warm_act, warm_sb[:, 0:1], ACTF.Exp)
        warm_ps = dpsum.tile([P, SUB], F32, tag="warm_ps")
        for _ in range(10):
            nc.tensor.matmul(
                warm_ps, lhsT=warm_sb[:, :P], rhs=warm_sb, start=True, stop=True
            )

        # ---- persistent input tiles (per-k so the first matmul only waits
        # on a 128KB slice, not the whole 1MB shard) ----
        a_sh = []
        b_sh = []
        for k in range(KC):
            ak = inp.tile([P, SH], dt_in, tag=f"ash{k}")
            bk = inp.tile([P, SH], dt_in, tag=f"bsh{k}")
            a_sh.append(ak)
            b_sh.append(bk)

        # full tensors as separate chunk tiles for fine-grained DMA deps.
        # b gets narrow leading chunks so the very first psum tile's inputs
        # land quickly; the bulk arrives in 2048-wide chunks.
        B_EDGES = [0, 1024, 2048, 3072, 4096]
        A_EDGES = [0, 2048, 4096]

        def chunked_alloc(name, edges):
            tiles = []
            for k in range(KC):
                row = []
                for ch in range(len(edges) - 1):
                    t = inp.tile(
                        [P, edges[ch + 1] - edges[ch]], dt_in, tag=f"{name}_{k}_{ch}"
                    )
                    row.append(t)
                tiles.append(row)
            return tiles

        b_f = chunked_alloc("bf", B_EDGES)
        a_f = chunked_alloc("af", A_EDGES)

        def chunk_of(edges, n0):
            for ch in range(len(edges) - 1):
                if n0 < edges[ch + 1]:
                    return ch, n0 - edges[ch]
            raise AssertionError

        # single ordered HWDGE queue: strict consumption order so the head
        # chunks get full HBM bandwidth (parallel queues steal BW from the
        # critical first blocks).
        for k in range(KC):
            nc.sync.dma_start(out=a_sh[k], in_=ats[k * P : (k + 1) * P, :])
        for ch in range(2):
            for k in range(KC):
                nc.sync.dma_start(
                    out=b_f[k][ch],
                    in_=btf[k * P : (k + 1) * P, B_EDGES[ch] : B_EDGES[ch + 1]],
                )
        for k in range(KC):
            nc.sync.dma_start(out=b_sh[k], in_=bts[k * P : (k + 1) * P, :])
        for ch in range(2, len(B_EDGES) - 1):
            for k in range(KC):
                nc.sync.dma_start(
                    out=b_f[k][ch],
                    in_=btf[k * P : (k + 1) * P, B_EDGES[ch] : B_EDGES[ch + 1]],
                )
        for ch in range(len(A_EDGES) - 1):
            for k in range(KC):
                nc.sync.dma_start(
                    out=a_f[k][ch],
                    in_=atf[k * P : (k + 1) * P, A_EDGES[ch] : A_EDGES[ch + 1]],
                )

        # diag prods on GpSimd (otherwise idle), emitted early so they are
        # long done before the diag ones-matmuls run (pinned after pass L0).
        prods = []
        for k in range(KC):
            prod = inp.tile([P, SH], dt_in, tag=f"prod{k}")
            nc.gpsimd.tensor_mul(prod, a_sh[k], b_sh[k])
            prods.append(prod)

        # raw per-block stats; the exact two-level LSE combine happens on the
        # host (removes Ln/table-load and all small fixup ops from the tail).
        negm_all = inp.tile([P, 2 * MT * NB], F32, tag="negm_all")
        sums_all = inp.tile([P, 2 * MT * NB], F32, tag="sums_all")

        # ---- diagonal: diag[i] = sum_d a_sh[d, i] * b_sh[d, i] ----
        # elementwise mul on VE, then partition-sum via a ones-matmul.
        ones = inp.tile([P, 1], dt_in, tag="ones")
        nc.vector.memset(ones, 1.0)
        # ---- main two passes ----
        from concourse.bass import _add_dep_helper

        def emit_diag(after_mm):
            dps = dpsum.tile([1, SH], F32)
            for k in range(KC):
                mm = nc.tensor.matmul(
                    dps, lhsT=ones, rhs=prods[k], start=(k == 0), stop=(k == KC - 1)
                )
                if k == 0 and after_mm is not None:
                    _add_dep_helper(
                        mm.ins, after_mm.ins, sync=False, reason="diag after L0"
                    )
            diag_sb = small.tile([1, SH], F32, tag="diag_sb")
            nc.scalar.copy(diag_sb, dps)
            nc.sync.dma_start(out=diag_out, in_=diag_sb)

        last_mm = None
        for L in range(2):
            if L == 1:
                emit_diag(last_mm)
            lhs = a_sh if L == 0 else b_sh
            rhs_t = b_f if L == 0 else a_f  # noqa
            edges = B_EDGES if L == 0 else A_EDGES
            # t outer / m inner: during the DMA ramp all MT psum tiles of a
            # given t consume the SAME 1024-wide rhs slice, so the PE extracts
            # 4x more work per DMA'd byte and never outruns HBM.
            for t in range(NB):
                for m in range(MT):
                    col = (L * MT + m) * NB + t
                    ps = psum.tile([P, BLK], F32, tag="ps")
                    for j in range(BLK // SUB):
                        n0 = t * BLK + j * SUB
                        chi, off = chunk_of(edges, n0)
                        for k in range(KC):
                            last_mm = nc.tensor.matmul(
                                ps[:, j * SUB : (j + 1) * SUB],
                                lhsT=lhs[k][:, m * P : (m + 1) * P],
                                rhs=rhs_t[k][chi][:, off : off + SUB],
                                start=(k == 0),
                                stop=(k == KC - 1),
                            )
                    # block stats straight from PSUM
                    nc.vector.reduce_max(
                        out=negm_all[:, col : col + 1], in_=ps, axis=AX.X, negate=True
                    )
                    sc = scr.tile([P, BLK], F32, tag="escr")
                    nc.scalar.activation(
                        sc,
                        ps,
                        ACTF.Exp,
                        bias=negm_all[:, col : col + 1],
                        accum_out=sums_all[:, col : col + 1],
                    )

        nc.sync.dma_start(out=negm_out, in_=negm_all)
        nc.sync.dma_start(out=sums_out, in_=sums_all)

    nc.compile()
    return nc


def _get_program(dt_in):
    key = str(dt_in)
    if key not in _prog_cache:
        _prog_cache[key] = _build_program(dt_in)
    return _prog_cache[key]


def kernel(out_ftir, out_raman, labels=None, log_tau=None, **_unused):
    global LAST_RESULTS
    out_ftir = np.asarray(out_ftir, dtype=np.float32)
    out_raman = np.asarray(out_raman, dtype=np.float32)
    tau = float(np.minimum(np.exp(np.float64(np.asarray(log_tau))), 100.0))

    np_dt = mybir.dt.np(DT_IN)
    aT = np.ascontiguousarray((out_ftir * np.float32(tau)).T).astype(np_dt)
    bT = np.ascontiguousarray(out_raman.T).astype(np_dt)

    in_maps = []
    for c in range(NCORES):
        sl = slice(c * SH, (c + 1) * SH)
        in_maps.append(
            {
                "ats": np.ascontiguousarray(aT[:, sl]),
                "bts": np.ascontiguousarray(bT[:, sl]),
                "atf": aT,
                "btf": bT,
            }
        )

    nc = _get_program(DT_IN)
    res = run_bass_kernel_spmd(
        nc, in_maps, core_ids=list(range(NCORES)), trace=PROFILE
    )
    LAST_RESULTS = res

    s_lse = 0.0
    s_diag = 0.0
    for r in res.results:
        # exact two-level LSE combine (float64):
        # LSE = M + log(sum_b s_b * exp(m_b - M)),  m_b = -negm
        mb = -r["negm"].astype(np.float64).reshape(P, 2 * MT, NB)
        sb = r["sums"].astype(np.float64).reshape(P, 2 * MT, NB)
        M = mb.max(axis=2, keepdims=True)
        lse = M[..., 0] + np.log((sb * np.exp(mb - M)).sum(axis=2))
        s_lse += float(lse.sum())
        s_diag += float(r["diag"].astype(np.float64).sum())
    loss = (s_lse - 2.0 * s_diag) / (2.0 * B)
    return np.array(loss, dtype=np.float32)



# revision 2
# speedup vs baseline: 1.2433x; 1.2433x over previous
"""Trainium2 Bass kernel for the distributed CLIP-style contrastive loss.

loss = 0.5 * ( mean_i( LSE_row(i) - diag(i) ) + mean_j( LSE_col(j) - diag(j) ) )
with logits = tau * ftir @ raman.T, tau = min(exp(log_tau), 100), B=4096, D=512.

Sharding: rows of the [B, B] logits matrix are split across 8 cores (512 rows
each).  Each core computes BOTH its row-slab of logits (ftir_shard @ raman.T)
and its row-slab of logits.T (raman_shard @ ftir.T), so the column-softmax is
just a second row-softmax and no collectives are needed.  Row log-sum-exp is
computed with an exact two-level scheme: per 1024-wide block the VectorE takes
the block max straight out of PSUM (negated, as the exp bias), the ScalarE
computes exp(x - m_b) with a fused free-dim accumulation (accum_out), and the
host combines block stats exactly: LSE = M + log(sum_b s_b * exp(m_b - M)).

Matmuls run in fp8 (e4m3, TRN flavor: max +-240) with perf_mode=DoubleRow,
which packs two fp8 weights per PE cell (virtual 128x256 array): each matmul
contracts K=256 in one instruction at ~2x the bf16 MAC rate.  Inputs are
laid out as [128, 2, n] tiles where dim1 selects the K-half (k, k+128).

Each core returns raw per-block stats (negm/sums, [128, 32]) and the diagonal
dot products ([1, 512]); the host does the exact two-level LSE combine and the
final scalar reduction in float64.
"""

import sys

import numpy as np

for _p in ("/opt/trn_rl_repo", "/root/.axon_site/_ro/trn_rl_repo"):
    if _p not in sys.path:
        sys.path.append(_p)

from contextlib import ExitStack

import concourse.bacc as bacc
import concourse.tile as tile
from concourse import mybir
from concourse.bass_utils import run_bass_kernel_spmd

B = 4096
D = 512
NCORES = 8
SH = B // NCORES  # 512 rows per core
P = 128
KS = D // 256  # 2 K-super-chunks of 256 (DoubleRow packs 2x128)
MT = SH // P  # 4 m-tiles of 128 rows
BLK = 1024  # PSUM stats-block width
NB = B // BLK  # 4 blocks per row
SUB = 512  # matmul N per instruction
CHW = 2048  # DMA chunk width for the full tensors

DT_IN = mybir.dt.float8e4
DT_DIAG = mybir.dt.bfloat16  # diag elementwise-product / ones-matmul dtype

F32 = mybir.dt.float32
AX = mybir.AxisListType
ALU = mybir.AluOpType
ACTF = mybir.ActivationFunctionType
PM_DR = mybir.MatmulPerfMode.DoubleRow

# toggled by test harness for profiling
PROFILE = False
LAST_RESULTS = None

_prog_cache = {}


def _build_program(dt_in):
    nc = bacc.Bacc(
        "TRN2",
        target_bir_lowering=False,
        debug=False,
        enable_partition_id=False,
        enable_asserts=False,
    )

    ats = nc.dram_tensor("ats", [D, SH], dt_in, kind="ExternalInput").ap()
    bts = nc.dram_tensor("bts", [D, SH], dt_in, kind="ExternalInput").ap()
    atf = nc.dram_tensor("atf", [D, B], dt_in, kind="ExternalInput").ap()
    btf = nc.dram_tensor("btf", [D, B], dt_in, kind="ExternalInput").ap()
    negm_out = nc.dram_tensor("negm", [P, 2 * MT * NB], F32, kind="ExternalOutput").ap()
    sums_out = nc.dram_tensor("sums", [P, 2 * MT * NB], F32, kind="ExternalOutput").ap()
    diag_out = nc.dram_tensor("diag", [1, SH], F32, kind="ExternalOutput").ap()

    with ExitStack() as ctx:
        tc = ctx.enter_context(tile.TileContext(nc))
        inp = ctx.enter_context(tc.tile_pool(name="inp", bufs=1))
        psum = ctx.enter_context(tc.tile_pool(name="psum", bufs=3, space="PSUM"))
        dpsum = ctx.enter_context(tc.tile_pool(name="dpsum", bufs=1, space="PSUM"))
        scr = ctx.enter_context(tc.tile_pool(name="scr", bufs=3))
        stats = ctx.enter_context(tc.tile_pool(name="stats", bufs=2))
        small = ctx.enter_context(tc.tile_pool(name="small", bufs=2))

        # ---- PE warm-up: dummy matmuls while input DMAs stream in. ----
        # Keeps TensorE busy through the DMA-bound head so HAM reaches
        # K=8/8 before the first real matmul (else ~25 MMs run at 1.2GHz).
        warm_sb = inp.tile([P, SUB], dt_in, tag="warm_sb")
        nc.vector.memset(warm_sb, 0.0)
        # dummy exp primes the ACT Exp table during the DMA-bound head —
        # otherwise the lazy ACT_TABLE_LOAD (1.28us) lands right before the
        # first real exp and delays the first PSUM release.
        warm_act = inp.tile([P, 1], F32, tag="warm_act")
        nc.scalar.activation(warm_act, warm_sb[:, 0:1], ACTF.Exp)
        warm_ps = dpsum.tile([P, SUB], F32, tag="warm_ps")
        for _ in range(10):
            nc.tensor.matmul(
                warm_ps, lhsT=warm_sb[:, :P], rhs=warm_sb, start=True, stop=True
            )

        # ---- persistent input tiles.  DoubleRow layout: [128, 2, n] where
        # dim1 = K-half (K rows s*256+h*128 .. +128).  Per-super so the first
        # matmul only waits on a small slice, not the whole shard. ----
        a_sh = []
        b_sh = []
        for s in range(KS):
            ak = inp.tile([P, 2, SH], dt_in, tag=f"ash{s}")
            bk = inp.tile([P, 2, SH], dt_in, tag=f"bsh{s}")
            a_sh.append(ak)
            b_sh.append(bk)

        # full tensors as separate chunk tiles for fine-grained DMA deps.
        # b gets narrow leading chunks so the very first psum tile's inputs
        # land quickly; the bulk arrives in 2048-wide chunks.
        B_EDGES = [0, 1024, 2048, 3072, 4096]
        A_EDGES = [0, 2048, 4096]

        def chunked_alloc(name, edges):
            tiles = []
            for s in range(KS):
                row = []
                for ch in range(len(edges) - 1):
                    t = inp.tile(
                        [P, 2, edges[ch + 1] - edges[ch]], dt_in, tag=f"{name}_{s}_{ch}"
                    )
                    row.append(t)
                tiles.append(row)
            return tiles

        b_f = chunked_alloc("bf", B_EDGES)
        a_f = chunked_alloc("af", A_EDGES)

        def chunk_of(edges, n0):
            for ch in range(len(edges) - 1):
                if n0 < edges[ch + 1]:
                    return ch, n0 - edges[ch]
            raise AssertionError

        # single ordered HWDGE queue: strict consumption order so the head
        # chunks get full HBM bandwidth (parallel queues steal BW from the
        # critical first blocks).
        for s in range(KS):
            for h in range(2):
                k0 = s * 256 + h * P
                nc.sync.dma_start(out=a_sh[s][:, h, :], in_=ats[k0 : k0 + P, :])
        for ch in range(2):
            for s in range(KS):
                for h in range(2):
                    k0 = s * 256 + h * P
                    nc.sync.dma_start(
                        out=b_f[s][ch][:, h, :],
                        in_=btf[k0 : k0 + P, B_EDGES[ch] : B_EDGES[ch + 1]],
                    )
        for s in range(KS):
            for h in range(2):
                k0 = s * 256 + h * P
                nc.sync.dma_start(out=b_sh[s][:, h, :], in_=bts[k0 : k0 + P, :])
        for ch in range(2, len(B_EDGES) - 1):
            for s in range(KS):
                for h in range(2):
                    k0 = s * 256 + h * P
                    nc.sync.dma_start(
                        out=b_f[s][ch][:, h, :],
                        in_=btf[k0 : k0 + P, B_EDGES[ch] : B_EDGES[ch + 1]],
                    )
        for ch in range(len(A_EDGES) - 1):
            for s in range(KS):
                for h in range(2):
                    k0 = s * 256 + h * P
                    nc.sync.dma_start(
                        out=a_f[s][ch][:, h, :],
                        in_=atf[k0 : k0 + P, A_EDGES[ch] : A_EDGES[ch + 1]],
                    )

        # diag prods on GpSimd (otherwise idle), emitted early so they are
        # long done before the diag ones-matmuls run (pinned after pass L0).
        # fp8 inputs, bf16 products (ones-matmul then runs as plain bf16).
        prods = []
        for s in range(KS):
            for h in range(2):
                prod = inp.tile([P, SH], DT_DIAG, tag=f"prod{s}_{h}")
                nc.gpsimd.tensor_mul(prod, a_sh[s][:, h, :], b_sh[s][:, h, :])
                prods.append(prod)

        # raw per-block stats; the exact two-level LSE combine happens on the
        # host (removes Ln/table-load and all small fixup ops from the tail).
        negm_all = inp.tile([P, 2 * MT * NB], F32, tag="negm_all")
        sums_all = inp.tile([P, 2 * MT * NB], F32, tag="sums_all")

        # ---- diagonal: diag[i] = sum_d a_sh[d, i] * b_sh[d, i] ----
        # elementwise mul on GpSimd, then partition-sum via a ones-matmul.
        ones = inp.tile([P, 1], DT_DIAG, tag="ones")
        nc.vector.memset(ones, 1.0)
        # ---- main two passes ----
        from concourse.bass import _add_dep_helper

        def emit_diag(after_mm):
            dps = dpsum.tile([1, SH], F32)
            for k in range(4):
                mm = nc.tensor.matmul(
                    dps, lhsT=ones, rhs=prods[k], start=(k == 0), stop=(k == 3)
                )
                if k == 0 and after_mm is not None:
                    _add_dep_helper(
                        mm.ins, after_mm.ins, sync=False, reason="diag after L0"
                    )
            diag_sb = small.tile([1, SH], F32, tag="diag_sb")
            nc.scalar.copy(diag_sb, dps)
            nc.sync.dma_start(out=diag_out, in_=diag_sb)

        last_mm = None
        for L in range(2):
            if L == 1:
                emit_diag(last_mm)
            lhs = a_sh if L == 0 else b_sh
            rhs_t = b_f if L == 0 else a_f  # noqa
            edges = B_EDGES if L == 0 else A_EDGES
            # t outer / m inner: during the DMA ramp all MT psum tiles of a
            # given t consume the SAME 1024-wide rhs slice, so the PE extracts
            # 4x more work per DMA'd byte and never outruns HBM.
            for t in range(NB):
                for m in range(MT):
                    col = (L * MT + m) * NB + t
                    ps = psum.tile([P, BLK], F32, tag="ps")
                    for j in range(BLK // SUB):
                        n0 = t * BLK + j * SUB
                        chi, off = chunk_of(edges, n0)
                        for s in range(KS):
                            last_mm = nc.tensor.matmul(
                                ps[:, j * SUB : (j + 1) * SUB],
                                lhsT=lhs[s][:, :, m * P : (m + 1) * P],
                                rhs=rhs_t[s][chi][:, :, off : off + SUB],
                                start=(s == 0),
                                stop=(s == KS - 1),
                                perf_mode=PM_DR,
                            )
                    # block stats straight from PSUM
                    nc.vector.reduce_max(
                        out=negm_all[:, col : col + 1], in_=ps, axis=AX.X, negate=True
                    )
                    sc = scr.tile([P, BLK], F32, tag="escr")
                    nc.scalar.activation(
                        sc,
                        ps,
                        ACTF.Exp,
                        bias=negm_all[:, col : col + 1],
                        accum_out=sums_all[:, col : col + 1],
                    )

        nc.sync.dma_start(out=negm_out, in_=negm_all)
        nc.sync.dma_start(out=sums_out, in_=sums_all)

    nc.compile()
    return nc


def _get_program(dt_in):
    key = str(dt_in)
    if key not in _prog_cache:
        _prog_cache[key] = _build_program(dt_in)
    return _prog_cache[key]


def kernel(out_ftir, out_raman, labels=None, log_tau=None, **_unused):
    global LAST_RESULTS
    out_ftir = np.asarray(out_ftir, dtype=np.float32)
    out_raman = np.asarray(out_raman, dtype=np.float32)
    tau = float(np.minimum(np.exp(np.float64(np.asarray(log_tau))), 100.0))

    np_dt = mybir.dt.np(DT_IN)
    aT = np.ascontiguousarray(
        np.clip((out_ftir * np.float32(tau)).T, -240.0, 240.0)
    ).astype(np_dt)
    bT = np.ascontiguousarray(np.clip(out_raman.T, -240.0, 240.0)).astype(np_dt)

    in_maps = []
    for c in range(NCORES):
        sl = slice(c * SH, (c + 1) * SH)
        in_maps.append(
            {
                "ats": np.ascontiguousarray(aT[:, sl]),
                "bts": np.ascontiguousarray(bT[:, sl]),
                "atf": aT,
                "btf": bT,
            }
        )

    nc = _get_program(DT_IN)
    res = run_bass_kernel_spmd(
        nc, in_maps, core_ids=list(range(NCORES)), trace=PROFILE
    )
    LAST_RESULTS = res

    s_lse = 0.0
    s_diag = 0.0
    for r in res.results:
        # exact two-level LSE combine (float64):
        # LSE = M + log(sum_b s_b * exp(m_b - M)),  m_b = -negm
        mb = -r["negm"].astype(np.float64).reshape(P, 2 * MT, NB)
        sb = r["sums"].astype(np.float64).reshape(P, 2 * MT, NB)
        M = mb.max(axis=2, keepdims=True)
        lse = M[..., 0] + np.log((sb * np.exp(mb - M)).sum(axis=2))
        s_lse += float(lse.sum())
        s_diag += float(r["diag"].astype(np.float64).sum())
    loss = (s_lse - 2.0 * s_diag) / (2.0 * B)
    return np.array(loss, dtype=np.float32)


# revision 10
# speedup vs baseline: 1.3068x; 1.0510x over previous
"""Trainium2 Bass kernel for the distributed CLIP-style contrastive loss.

loss = 0.5 * ( mean_i( LSE_row(i) - diag(i) ) + mean_j( LSE_col(j) - diag(j) ) )
with logits = tau * ftir @ raman.T, tau = min(exp(log_tau), 100), B=4096, D=512.

Sharding: rows of the [B, B] logits matrix are split across 8 cores (512 rows
each).  Each core computes BOTH its row-slab of logits (ftir_shard @ raman.T)
and its row-slab of logits.T (raman_shard @ ftir.T), so the column-softmax is
just a second row-softmax and no collectives are needed.  Row log-sum-exp is
computed with an exact two-level scheme: per 1024-wide block the VectorE takes
the block max straight out of PSUM (negated, as the exp bias), the ScalarE
computes exp(x - m_b) with a fused free-dim accumulation (accum_out), and the
host combines block stats exactly: LSE = M + log(sum_b s_b * exp(m_b - M)).

Matmuls run in fp8 (e4m3, TRN flavor: max +-240) with perf_mode=DoubleRow,
which packs two fp8 weights per PE cell (virtual 128x256 array): each matmul
contracts K=256 in one instruction at ~2x the bf16 MAC rate.  Inputs are
laid out as [128, 2, n] tiles where dim1 selects the K-half (k, k+128).

Each core returns raw per-block stats (negm/sums, [128, 32]) and the diagonal
dot products ([1, 512]); the host does the exact two-level LSE combine and the
final scalar reduction in float64.
"""

import sys

import numpy as np

for _p in ("/opt/trn_rl_repo", "/root/.axon_site/_ro/trn_rl_repo"):
    if _p not in sys.path:
        sys.path.append(_p)

from contextlib import ExitStack

import concourse.bacc as bacc
import concourse.tile as tile
from concourse import mybir
from concourse.bass_utils import run_bass_kernel_spmd

B = 4096
D = 512
NCORES = 8
SH = B // NCORES  # 512 rows per core
P = 128
KS = D // 256  # 2 K-super-chunks of 256 (DoubleRow packs 2x128)
MT = SH // P  # 4 m-tiles of 128 rows
BLK = 1024  # PSUM stats-block width
NB = B // BLK  # 4 blocks per row
SUB = 512  # matmul N per instruction
CHW = 2048  # DMA chunk width for the full tensors

DT_IN = mybir.dt.float8e4
DT_SCR = mybir.dt.bfloat16  # exp scratch output dtype (value is discarded)

F32 = mybir.dt.float32
AX = mybir.AxisListType
ALU = mybir.AluOpType
ACTF = mybir.ActivationFunctionType
PM_DR = mybir.MatmulPerfMode.DoubleRow

# toggled by test harness for profiling
PROFILE = False
LAST_RESULTS = None

_prog_cache = {}


def _build_program(dt_in):
    nc = bacc.Bacc(
        "TRN2",
        target_bir_lowering=False,
        debug=False,
        enable_partition_id=False,
        enable_asserts=False,
    )

    ats = nc.dram_tensor("ats", [D, SH], dt_in, kind="ExternalInput").ap()
    bts = nc.dram_tensor("bts", [D, SH], dt_in, kind="ExternalInput").ap()
    atf = nc.dram_tensor("atf", [D, B], dt_in, kind="ExternalInput").ap()
    btf = nc.dram_tensor("btf", [D, B], dt_in, kind="ExternalInput").ap()
    negm_out = nc.dram_tensor("negm", [P, 2 * MT * NB], F32, kind="ExternalOutput").ap()
    sums_out = nc.dram_tensor("sums", [P, 2 * MT * NB], F32, kind="ExternalOutput").ap()

    with ExitStack() as ctx:
        tc = ctx.enter_context(tile.TileContext(nc))
        inp = ctx.enter_context(tc.tile_pool(name="inp", bufs=1))
        psum = ctx.enter_context(tc.tile_pool(name="psum", bufs=4, space="PSUM"))
        scr = ctx.enter_context(tc.tile_pool(name="scr", bufs=3))

        # ---- PE warm-up: a couple of dummy matmuls while input DMAs stream
        # in, so the PE pipeline/pstate is past the cold state when the first
        # real matmul issues.  Stats (not PE) are the critical path, so a
        # short warm-up that lets real blocks start ~6us earlier beats a long
        # one that reaches full clock before any real work.
        warm_sb = inp.tile([P, SUB], dt_in, tag="warm_sb")
        nc.vector.memset(warm_sb, 0.0)
        # dummy exp primes the ACT Exp table during the DMA-bound head —
        # otherwise the lazy ACT_TABLE_LOAD (1.28us) lands right before the
        # first real exp and delays the first PSUM release.
        warm_act = inp.tile([P, 1], F32, tag="warm_act")
        nc.scalar.activation(warm_act, warm_sb[:, 0:1], ACTF.Exp)
        warm_ps = psum.tile([P, BLK], F32, tag="ps")
        for _ in range(2):
            nc.tensor.matmul(
                warm_ps[:, :SUB], lhsT=warm_sb[:, :P], rhs=warm_sb, start=True, stop=True
            )

        # ---- persistent input tiles.  DoubleRow layout: [128, 2, n] where
        # dim1 = K-half (K rows s*256+h*128 .. +128).  Per-super so the first
        # matmul only waits on a small slice, not the whole shard. ----
        a_sh = []
        b_sh = []
        for s in range(KS):
            ak = inp.tile([P, 2, SH], dt_in, tag=f"ash{s}")
            bk = inp.tile([P, 2, SH], dt_in, tag=f"bsh{s}")
            a_sh.append(ak)
            b_sh.append(bk)

        # full tensors as separate chunk tiles for fine-grained DMA deps.
        # b gets narrow leading chunks so the very first psum tile's inputs
        # land quickly; the bulk arrives in 2048-wide chunks.
        B_EDGES = [0, 1024, 2048, 3072, 4096]
        A_EDGES = [0, 2048, 4096]

        def chunked_alloc(name, edges):
            tiles = []
            for s in range(KS):
                row = []
                for ch in range(len(edges) - 1):
                    t = inp.tile(
                        [P, 2, edges[ch + 1] - edges[ch]], dt_in, tag=f"{name}_{s}_{ch}"
                    )
                    row.append(t)
                tiles.append(row)
            return tiles

        b_f = chunked_alloc("bf", B_EDGES)
        a_f = chunked_alloc("af", A_EDGES)

        def chunk_of(edges, n0):
            for ch in range(len(edges) - 1):
                if n0 < edges[ch + 1]:
                    return ch, n0 - edges[ch]
            raise AssertionError

        # single ordered HWDGE queue: strict consumption order so the head
        # chunks get full HBM bandwidth (parallel queues steal BW from the
        # critical first blocks).
        for s in range(KS):
            for h in range(2):
                k0 = s * 256 + h * P
                nc.sync.dma_start(out=a_sh[s][:, h, :], in_=ats[k0 : k0 + P, :])
        for ch in range(2):
            for s in range(KS):
                for h in range(2):
                    k0 = s * 256 + h * P
                    nc.sync.dma_start(
                        out=b_f[s][ch][:, h, :],
                        in_=btf[k0 : k0 + P, B_EDGES[ch] : B_EDGES[ch + 1]],
                    )
        for s in range(KS):
            for h in range(2):
                k0 = s * 256 + h * P
                nc.sync.dma_start(out=b_sh[s][:, h, :], in_=bts[k0 : k0 + P, :])
        for ch in range(2, len(B_EDGES) - 1):
            for s in range(KS):
                for h in range(2):
                    k0 = s * 256 + h * P
                    nc.sync.dma_start(
                        out=b_f[s][ch][:, h, :],
                        in_=btf[k0 : k0 + P, B_EDGES[ch] : B_EDGES[ch + 1]],
                    )
        for ch in range(len(A_EDGES) - 1):
            for s in range(KS):
                for h in range(2):
                    k0 = s * 256 + h * P
                    nc.sync.dma_start(
                        out=a_f[s][ch][:, h, :],
                        in_=atf[k0 : k0 + P, A_EDGES[ch] : A_EDGES[ch + 1]],
                    )

        # raw per-block stats; the exact two-level LSE combine happens on the
        # host (removes Ln/table-load and all small fixup ops from the tail).
        # The diagonal dot products are also computed host-side from the same
        # quantized fp8 inputs, freeing PSUM bank 8 and the GpSimd/ones path.
        negm_all = inp.tile([P, 2 * MT * NB], F32, tag="negm_all")
        sums_all = inp.tile([P, 2 * MT * NB], F32, tag="sums_all")

        # ---- main two passes ----
        for L in range(2):
            lhs = a_sh if L == 0 else b_sh
            rhs_t = b_f if L == 0 else a_f  # noqa
            edges = B_EDGES if L == 0 else A_EDGES
            # t outer / m inner: during the DMA ramp all MT psum tiles of a
            # given t consume the SAME 1024-wide rhs slice, so the PE extracts
            # 4x more work per DMA'd byte and never outruns HBM.
            for t in range(NB):
                for m in range(MT):
                    col = (L * MT + m) * NB + t
                    ps = psum.tile([P, BLK], F32, tag="ps")
                    for j in range(BLK // SUB):
                        n0 = t * BLK + j * SUB
                        chi, off = chunk_of(edges, n0)
                        for s in range(KS):
                            nc.tensor.matmul(
                                ps[:, j * SUB : (j + 1) * SUB],
                                lhsT=lhs[s][:, :, m * P : (m + 1) * P],
                                rhs=rhs_t[s][chi][:, :, off : off + SUB],
                                start=(s == 0),
                                stop=(s == KS - 1),
                                perf_mode=PM_DR,
                            )
                    # block stats straight from PSUM
                    nc.vector.reduce_max(
                        out=negm_all[:, col : col + 1], in_=ps, axis=AX.X, negate=True
                    )
                    sc = scr.tile([P, BLK], DT_SCR, tag="escr")
                    nc.scalar.activation(
                        sc,
                        ps,
                        ACTF.Exp,
                        bias=negm_all[:, col : col + 1],
                        accum_out=sums_all[:, col : col + 1],
                    )

        nc.sync.dma_start(out=negm_out, in_=negm_all)
        nc.sync.dma_start(out=sums_out, in_=sums_all)

    nc.compile()
    return nc


def _get_program(dt_in):
    key = str(dt_in)
    if key not in _prog_cache:
        _prog_cache[key] = _build_program(dt_in)
    return _prog_cache[key]


def kernel(out_ftir, out_raman, labels=None, log_tau=None, **_unused):
    global LAST_RESULTS
    out_ftir = np.asarray(out_ftir, dtype=np.float32)
    out_raman = np.asarray(out_raman, dtype=np.float32)
    tau = float(np.minimum(np.exp(np.float64(np.asarray(log_tau))), 100.0))

    np_dt = mybir.dt.np(DT_IN)
    aT = np.ascontiguousarray(
        np.clip((out_ftir * np.float32(tau)).T, -240.0, 240.0)
    ).astype(np_dt)
    bT = np.ascontiguousarray(np.clip(out_raman.T, -240.0, 240.0)).astype(np_dt)

    # diagonal logits host-side from the same quantized values the device
    # matmuls consume: diag[i] = sum_d aT[d,i] * bT[d,i]
    diag_full = np.einsum(
        "di,di->i",
        aT.astype(np.float32),
        bT.astype(np.float32),
        dtype=np.float64,
    )

    in_maps = []
    for c in range(NCORES):
        sl = slice(c * SH, (c + 1) * SH)
        in_maps.append(
            {
                "ats": np.ascontiguousarray(aT[:, sl]),
                "bts": np.ascontiguousarray(bT[:, sl]),
                "atf": aT,
                "btf": bT,
            }
        )

    nc = _get_program(DT_IN)
    res = run_bass_kernel_spmd(
        nc, in_maps, core_ids=list(range(NCORES)), trace=PROFILE
    )
    LAST_RESULTS = res

    s_lse = 0.0
    for r in res.results:
        # exact two-level LSE combine (float64):
        # LSE = M + log(sum_b s_b * exp(m_b - M)),  m_b = -negm
        mb = -r["negm"].astype(np.float64).reshape(P, 2 * MT, NB)
        sb = r["sums"].astype(np.float64).reshape(P, 2 * MT, NB)
        M = mb.max(axis=2, keepdims=True)
        lse = M[..., 0] + np.log((sb * np.exp(mb - M)).sum(axis=2))
        s_lse += float(lse.sum())
    s_diag = float(diag_full.sum())
    loss = (s_lse - 2.0 * s_diag) / (2.0 * B)
    return np.array(loss, dtype=np.float32)
